# revision 3
# baseline (speedup 1.0000x reference)
"""Full attention (B=4, L=S=2048, H=16, E=D=64, fp32) on 8 TRN2 NeuronCores.

Sharding: the 64 (batch, head) pairs are split 8-per-core (data + head
parallel); each core runs full attention for its heads with no cross-core
communication. The host pre-arranges all layouts so the device needs no
transposes:
  qt/kt: per-head Q^T/K^T as [E, L] bf16 (l contiguous)
  vt:    per-head [V | ones] s-chunk-transposed to [128, chunk*65] bf16
  out:   per-head O^T as [D, L] f32 (host transposes back)

Device algorithm per head (ScalarE-exp-throughput bound, ~1163 ns per
[128,1024] score chunk):
  - S^T chunk [s=128, l=1024] = matmul(lhsT=K^T[e, s-chunk], rhs=Q^T[e, l])
    as bf16 with fp32 PSUM. The e-contraction is zero-padded 64->128 so
    EVERY matmul runs the same (128,128) PE config: alternating 64-row and
    128-row configs makes each matmul pay a full array drain (~2x slower).
    bf16 moving operands stream at 1 col/cycle; fp32/fp32r stream at half
    rate, which is why operands are bf16 (PSUM accumulation stays fp32).
  - exp on ScalarE reads the PSUM scores directly, writes bf16 to SBUF,
    with the 1/sqrt(E) scale folded into the activation pre-scale. No max
    subtraction: scaled scores are ~N(0,1), far inside fp32 range.
  - U^T[65, l] += matmul(lhsT=[V|1][s-chunk, 65], rhs=exp(S^T)) accumulated
    over the 16 s-chunks in PSUM; row 64 (ones column) is the softmax
    denominator Z for free.
  - out[d, l] = U^T[d, l] * (1/Z[l]): DVE copy of Z to SBUF (the custom-DVE
    fast reciprocal misreads PSUM sources), reciprocal_approx_fast, gpsimd
    partition_broadcast, DVE multiply.

The very last tile's normalization is split into two 512-column
half-chains with interleaved emission (Z-copies on the now-idle ScalarE
and reciprocals first, then broadcast+multiply+store per half) so the
serial tail chain overlaps across the Scalar, Vector and GpSimd engines
and the final stores ride the idle HWDGE queue. Head 0's first-needed
Q^T half loads via the second HWDGE engine (scalar) so the two initial
loads' HBM completion latencies overlap.

Emission is software-pipelined with a TWO-slot lookahead — chunk t+2's
MM1s are emitted before chunk t's exp+MM2s. In the PE's in-order FIFO
every MM1 then precedes the MM2 that would otherwise block it, so each
MM1 fires the moment its PSUM buffer frees and finishes a full chunk
before its exp needs it. This makes the 256 ScalarE exps run perfectly
back-to-back (cadence = slice = ~1005 ns, ~1 us total idle): the kernel
sits exactly on the ScalarE exp-throughput floor for 1024-wide
instructions. l is processed in 1024-wide halves so PSUM holds
double-buffered score tiles (2x2 banks) plus the double-buffered U^T
accumulator (2x2 banks). Input loads are HWDGE (sync) DMAs prefetched one
head ahead; memsets (zero-padding) ride gpsimd; output stores ride the
gpsimd SWDGE queue so their semaphore waits never block input prefetch.
"""

import numpy as np

B, L, S, H, E, D = 4, 2048, 2048, 16, 64, 64
N_CORES = 8
HPC = (B * H) // N_CORES
NCH = S // 128
LG = 2
LW = L // LG
NG = LW // 512
VW = D + 1

_compiled = None

# Chunk positions (within each lg's 16 s-chunks) whose exp runs on the DVE
# as a Schraudolph bit-trick instead of the ScalarE ACT table: the affine
# i16 = trunc(EXPA*score + EXPB) yields the bf16 BIT PATTERN of
# exp(0.125*score) (max rel err ~3%, RMS ~1.8%). With the softmax
# denominator absorbing the common-mode error, offloading D of 256 chunks
# contributes ~sqrt(D/256)*1.8% output error - well under the 2e-2 gate -
# while removing D*996ns from the ScalarE exp floor (the kernel's
# bottleneck). Each DVE chunk overlaps the next ScalarE chunk (the two
# PSUM score slots feed the two consumers concurrently).
DVE_CHUNKS = (2, 7, 12)
EXPA = 0.125 * 1.4426950408889634 * 128.0   # scale * log2(e) * 2^7
EXPB = 16256.0 - 0.05 * 128.0               # (127 - C)*2^7, C=0.05


def _build():
    import concourse.tile as tile
    from concourse import bacc, mybir

    f32 = mybir.dt.float32
    bf16 = mybir.dt.bfloat16
    i16 = mybir.dt.int16
    Exp = mybir.ActivationFunctionType.Exp
    Mult = mybir.AluOpType.mult
    Add = mybir.AluOpType.add

    nc = bacc.Bacc("TRN2", target_bir_lowering=False, debug=False,
                   enable_asserts=False)
    qt = nc.declare_dram_parameter("qt", [HPC * E, L], bf16, isOutput=False)
    kt = nc.declare_dram_parameter("kt", [HPC * E, S], bf16, isOutput=False)
    vt = nc.declare_dram_parameter("vt", [HPC * 128, NCH * VW], bf16,
                                   isOutput=False)
    out = nc.declare_dram_parameter("out", [HPC * D, L], f32, isOutput=True)

    with tile.TileContext(nc) as tc:
        with (
            tc.tile_pool(name="qk", bufs=2) as qk_pool,
            tc.tile_pool(name="vtp", bufs=2) as vt_pool,
            tc.tile_pool(name="exp", bufs=3) as exp_pool,
            tc.tile_pool(name="osb", bufs=2) as o_pool,
            tc.tile_pool(name="nrm", bufs=2) as nrm_pool,
            tc.tile_pool(name="ps_s", bufs=2, space="PSUM") as ps_s_pool,
            tc.tile_pool(name="ps_o", bufs=2, space="PSUM") as ps_o_pool,
        ):
            heads = {}   # head -> (qt_t, kt_t, vt_t, o_t)
            psos = {}    # (head, lg) -> ps_o tile

            def load_head(head):
                qt_t = qk_pool.tile([128, L], bf16, name="qt_t", tag="qt")
                kt_t = qk_pool.tile([128, S], bf16, name="kt_t", tag="kt")
                vt_t = vt_pool.tile([128, NCH * VW], bf16, name="vt_t",
                                    tag="vt")
                nc.sync.dma_start(
                    out=kt_t[0:E, 0:128],
                    in_=kt.ap()[head * E:(head + 1) * E, 0:128])
                nc.gpsimd.memset(kt_t[E:128, 0:128], 0.0)
                # head 0's first-needed qt half rides the second HWDGE engine
                # (ACT queue is empty before the first exp) so the two loads'
                # ~2us HBM completion latencies overlap
                qeng = nc.scalar if head == 0 else nc.sync
                qeng.dma_start(
                    out=qt_t[0:E, 0:LW],
                    in_=qt.ap()[head * E:(head + 1) * E, 0:LW])
                nc.gpsimd.memset(qt_t[E:128, 0:LW], 0.0)
                nc.sync.dma_start(
                    out=kt_t[0:E, 128:S],
                    in_=kt.ap()[head * E:(head + 1) * E, 128:S])
                nc.gpsimd.memset(kt_t[E:128, 128:S], 0.0)
                nc.sync.dma_start(
                    out=qt_t[0:E, LW:L],
                    in_=qt.ap()[head * E:(head + 1) * E, LW:L])
                nc.gpsimd.memset(qt_t[E:128, LW:L], 0.0)
                nc.sync.dma_start(
                    out=vt_t[:, :],
                    in_=vt.ap()[head * 128:(head + 1) * 128, :])
                o_t = o_pool.tile([64, L], f32, name="o_t", tag="o")
                heads[head] = (qt_t, kt_t, vt_t, o_t)

            def emit_mm1(head, lg, i):
                if lg == 0 and i == 0 and head not in heads:
                    load_head(head)
                if lg == 1 and i == 8 and head + 1 < HPC:
                    load_head(head + 1)
                if i == 0:
                    psos[(head, lg)] = ps_o_pool.tile(
                        [VW, LW], f32, name="ps_o", tag="ps_o")
                qt_t, kt_t, _, _ = heads[head]
                ps_s = ps_s_pool.tile([128, LW], f32, name="ps_s", tag="ps_s")
                for g in range(NG):
                    nc.tensor.matmul(
                        out=ps_s[:, g * 512:(g + 1) * 512],
                        lhsT=kt_t[:, i * 128:(i + 1) * 128],
                        rhs=qt_t[:, lg * LW + g * 512:lg * LW + (g + 1) * 512],
                        start=True, stop=True)
                return ps_s

            def emit_tail(head, lg, i, ps_s):
                qt_t, kt_t, vt_t, o_t = heads[head]
                ps_o = psos[(head, lg)]
                e_t = exp_pool.tile([128, LW], bf16, name="e_t", tag="e_t")
                if i in DVE_CHUNKS:
                    nc.vector.tensor_scalar(
                        out=e_t[:, :].bitcast(i16),
                        in0=ps_s[:, :],
                        scalar1=EXPA,
                        scalar2=EXPB,
                        op0=Mult,
                        op1=Add,
                    )
                else:
                    nc.scalar.activation(e_t[:, :], ps_s[:, :], Exp,
                                         scale=0.125)
                for g in range(NG):
                    nc.tensor.matmul(
                        out=ps_o[:, g * 512:(g + 1) * 512],
                        lhsT=vt_t[:, i * VW:(i + 1) * VW],
                        rhs=e_t[:, g * 512:(g + 1) * 512],
                        start=(i == 0), stop=(i == NCH - 1))
                if i == NCH - 1:
                    final = (head == HPC - 1 and lg == LG - 1)
                    if not final:
                        zc_t = nrm_pool.tile([1, LW], f32, name="zc",
                                             tag="zc")
                        nc.vector.tensor_copy(zc_t[:, :], ps_o[64:65, :])
                        recip_t = nrm_pool.tile([1, LW], f32, name="re",
                                                tag="recip")
                        nc.vector.reciprocal_approx_fast(recip_t[:, :],
                                                         zc_t[:, :])
                        bcast_t = nrm_pool.tile([64, LW], f32, name="bc",
                                                tag="bcast")
                        nc.gpsimd.partition_broadcast(bcast_t[:, :],
                                                      recip_t[:, :],
                                                      channels=64)
                        nc.vector.tensor_mul(o_t[:, lg * LW:(lg + 1) * LW],
                                             ps_o[0:64, :], bcast_t[:, :])
                        nc.gpsimd.dma_start(
                            out=out.ap()[head * 64:(head + 1) * 64,
                                         lg * LW:(lg + 1) * LW],
                            in_=o_t[:, lg * LW:(lg + 1) * LW])
                        return
                    # final tile: split into 512-col half-chains, emission
                    # interleaved so the DVE FIFO never blocks on gpsimd
                    halves = ((0, 512), (512, LW))
                    rts = []
                    for p, (c0, c1) in enumerate(halves):
                        w = c1 - c0
                        zc_t = nrm_pool.tile([1, w], f32, name=f"fzc{p}",
                                             tag=f"fzc{p}")
                        nc.scalar.copy(zc_t[:, :], ps_o[64:65, c0:c1])
                        recip_t = nrm_pool.tile([1, w], f32, name=f"fre{p}",
                                                tag=f"fre{p}")
                        nc.vector.reciprocal_approx_fast(recip_t[:, :],
                                                         zc_t[:, :])
                        rts.append(recip_t)
                    for p, (c0, c1) in enumerate(halves):
                        w = c1 - c0
                        bcast_t = nrm_pool.tile([64, w], f32, name=f"fbc{p}",
                                                tag=f"fbc{p}")
                        nc.gpsimd.partition_broadcast(bcast_t[:, :],
                                                      rts[p][:, :],
                                                      channels=64)
                        nc.vector.tensor_mul(
                            o_t[:, lg * LW + c0:lg * LW + c1],
                            ps_o[0:64, c0:c1], bcast_t[:, :])
                        nc.sync.dma_start(
                            out=out.ap()[head * 64:(head + 1) * 64,
                                         lg * LW + c0:lg * LW + c1],
                            in_=o_t[:, lg * LW + c0:lg * LW + c1])

            slots = [(head, lg, i)
                     for head in range(HPC)
                     for lg in range(LG)
                     for i in range(NCH)]
            # head 0's loads FIRST: the scalar-engine qt DMA must issue
            # before the warm exp's ACT table load occupies that queue
            load_head(0)
            # warm the ACT exp table set during the load ramp
            warm_t = nrm_pool.tile([1, 8], f32, tag="warm")
            nc.vector.memset(warm_t[:, :], 0.0)
            nc.scalar.activation(warm_t[:, :], warm_t[:, :], Exp, scale=1.0)

            pend = []
            for head, lg, i in slots:
                ps_s = emit_mm1(head, lg, i)
                pend.append((head, lg, i, ps_s))
                if len(pend) > 2:
                    emit_tail(*pend.pop(0))
            while pend:
                emit_tail(*pend.pop(0))
    nc.compile()
    return nc


def _prep_inputs(queries, keys, values):
    import ml_dtypes

    bf = ml_dtypes.bfloat16
    q = np.asarray(queries, dtype=np.float32)
    k = np.asarray(keys, dtype=np.float32)
    v = np.asarray(values, dtype=np.float32)
    BH = B * H
    qt = np.ascontiguousarray(q.transpose(0, 2, 3, 1)).astype(bf).reshape(
        BH, E, L)
    kt = np.ascontiguousarray(k.transpose(0, 2, 3, 1)).astype(bf).reshape(
        BH, E, S)
    vp = np.concatenate([v, np.ones((B, S, H, 1), np.float32)], axis=3)
    vt = (np.ascontiguousarray(
            vp.transpose(0, 2, 1, 3)
              .reshape(BH, NCH, 128, VW)
              .transpose(0, 2, 1, 3))
          .astype(bf)
          .reshape(BH, 128, NCH * VW))
    in_maps = []
    for c in range(N_CORES):
        sl = slice(c * HPC, (c + 1) * HPC)
        in_maps.append({
            "qt": np.ascontiguousarray(qt[sl]).reshape(HPC * E, L),
            "kt": np.ascontiguousarray(kt[sl]).reshape(HPC * E, S),
            "vt": np.ascontiguousarray(vt[sl]).reshape(HPC * 128, NCH * VW),
        })
    return in_maps


def _run(queries, keys, values, trace=False):
    global _compiled
    from concourse.bass_utils import run_bass_kernel_spmd

    if _compiled is None:
        _compiled = _build()
    in_maps = _prep_inputs(queries, keys, values)
    res = run_bass_kernel_spmd(_compiled, in_maps,
                               core_ids=list(range(N_CORES)), trace=trace)
    outs = np.stack([res.results[c]["out"] for c in range(N_CORES)])
    full = (outs.reshape(B * H, D, L)
                .reshape(B, H, D, L)
                .transpose(0, 3, 1, 2))
    return np.ascontiguousarray(full), res.exec_time_ns


def kernel(queries, keys, values):
    out, _ = _run(queries, keys, values, trace=False)
    return out



# revision 9
# speedup vs baseline: 1.0120x; 1.0120x over previous
"""Full attention (B=4, L=S=2048, H=16, E=D=64, fp32) on 8 TRN2 NeuronCores.

Sharding: the 64 (batch, head) pairs are split 8-per-core (data + head
parallel); each core runs full attention for its heads with no cross-core
communication. The host pre-arranges all layouts so the device needs no
transposes:
  qt/kt: per-head Q^T/K^T as [E, L] bf16 (l contiguous)
  vt:    per-head [V | ones] s-chunk-transposed to [128, chunk*65] bf16
  out:   per-head O^T as [D, L] f32 (host transposes back)

Device algorithm per head (ScalarE-exp-throughput bound, ~1163 ns per
[128,1024] score chunk):
  - S^T chunk [s=128, l=1024] = matmul(lhsT=K^T[e, s-chunk], rhs=Q^T[e, l])
    as bf16 with fp32 PSUM. The e-contraction is zero-padded 64->128 so
    EVERY matmul runs the same (128,128) PE config: alternating 64-row and
    128-row configs makes each matmul pay a full array drain (~2x slower).
    bf16 moving operands stream at 1 col/cycle; fp32/fp32r stream at half
    rate, which is why operands are bf16 (PSUM accumulation stays fp32).
  - exp on ScalarE reads the PSUM scores directly, writes bf16 to SBUF,
    with the 1/sqrt(E) scale folded into the activation pre-scale. No max
    subtraction: scaled scores are ~N(0,1), far inside fp32 range.
  - U^T[65, l] += matmul(lhsT=[V|1][s-chunk, 65], rhs=exp(S^T)) accumulated
    over the 16 s-chunks in PSUM; row 64 (ones column) is the softmax
    denominator Z for free.
  - out[d, l] = U^T[d, l] * (1/Z[l]): DVE copy of Z to SBUF (the custom-DVE
    fast reciprocal misreads PSUM sources), reciprocal_approx_fast, gpsimd
    partition_broadcast, DVE multiply.

The very last tile's normalization is split into two 512-column
half-chains with interleaved emission (Z-copies on the now-idle ScalarE
and reciprocals first, then broadcast+multiply+store per half) so the
serial tail chain overlaps across the Scalar, Vector and GpSimd engines
and the final stores ride the idle HWDGE queue. Head 0's first-needed
Q^T half loads via the second HWDGE engine (scalar) so the two initial
loads' HBM completion latencies overlap.

Emission is software-pipelined with a TWO-slot lookahead — chunk t+2's
MM1s are emitted before chunk t's exp+MM2s. In the PE's in-order FIFO
every MM1 then precedes the MM2 that would otherwise block it, so each
MM1 fires the moment its PSUM buffer frees and finishes a full chunk
before its exp needs it. This makes the 256 ScalarE exps run perfectly
back-to-back (cadence = slice = ~1005 ns, ~1 us total idle): the kernel
sits exactly on the ScalarE exp-throughput floor for 1024-wide
instructions. l is processed in 1024-wide halves so PSUM holds
double-buffered score tiles (2x2 banks) plus the double-buffered U^T
accumulator (2x2 banks). Input loads are HWDGE (sync) DMAs prefetched one
head ahead; memsets (zero-padding) ride gpsimd; output stores ride the
gpsimd SWDGE queue so their semaphore waits never block input prefetch.
"""

import numpy as np

B, L, S, H, E, D = 4, 2048, 2048, 16, 64, 64
N_CORES = 8
HPC = (B * H) // N_CORES
NCH = S // 128
LG = 2
LW = L // LG
NG = LW // 512
VW = D + 1

_compiled = None

# Chunk positions (within each lg's 16 s-chunks) whose exp runs on the DVE
# as a Schraudolph bit-trick instead of the ScalarE ACT table: the affine
# i16 = trunc(EXPA*score + EXPB) yields the bf16 BIT PATTERN of
# exp(0.125*score) (max rel err ~3%, RMS ~1.8%). With the softmax
# denominator absorbing the common-mode error, offloading D of 256 chunks
# contributes ~sqrt(D/256)*1.8% output error - well under the 2e-2 gate -
# while removing D*996ns from the ScalarE exp floor (the kernel's
# bottleneck). Each DVE chunk overlaps the next ScalarE chunk (the two
# PSUM score slots feed the two consumers concurrently).
DVE_CHUNKS = (6, 10, 14)
EXPA = 0.125 * 1.4426950408889634 * 128.0   # scale * log2(e) * 2^7
EXPB = 16256.0 - 0.05 * 128.0               # (127 - C)*2^7, C=0.05


def _build():
    import concourse.tile as tile
    from concourse import bacc, mybir

    f32 = mybir.dt.float32
    bf16 = mybir.dt.bfloat16
    i16 = mybir.dt.int16
    Exp = mybir.ActivationFunctionType.Exp
    Mult = mybir.AluOpType.mult
    Add = mybir.AluOpType.add

    nc = bacc.Bacc("TRN2", target_bir_lowering=False, debug=False,
                   enable_asserts=False)
    qt = nc.declare_dram_parameter("qt", [HPC * E, L], bf16, isOutput=False)
    kt = nc.declare_dram_parameter("kt", [HPC * E, S], bf16, isOutput=False)
    vt = nc.declare_dram_parameter("vt", [HPC * 128, NCH * VW], bf16,
                                   isOutput=False)
    out = nc.declare_dram_parameter("out", [HPC * D, L], f32, isOutput=True)

    with tile.TileContext(nc) as tc:
        with (
            tc.tile_pool(name="qk", bufs=2) as qk_pool,
            tc.tile_pool(name="vtp", bufs=2) as vt_pool,
            tc.tile_pool(name="exp", bufs=3) as exp_pool,
            tc.tile_pool(name="osb", bufs=2) as o_pool,
            tc.tile_pool(name="nrm", bufs=2) as nrm_pool,
            tc.tile_pool(name="ps_s", bufs=3, space="PSUM") as ps_s_pool,
            tc.tile_pool(name="ps_o", bufs=1, space="PSUM") as ps_o_pool,
        ):
            heads = {}   # head -> (qt_t, kt_t, vt_t, o_t)
            psos = {}    # (head, lg) -> ps_o tile

            def load_head(head):
                qt_t = qk_pool.tile([128, L], bf16, name="qt_t", tag="qt")
                kt_t = qk_pool.tile([128, S], bf16, name="kt_t", tag="kt")
                vt_t = vt_pool.tile([128, NCH * VW], bf16, name="vt_t",
                                    tag="vt")
                nc.sync.dma_start(
                    out=kt_t[0:E, 0:128],
                    in_=kt.ap()[head * E:(head + 1) * E, 0:128])
                nc.gpsimd.memset(kt_t[E:128, 0:128], 0.0)
                # head 0's first-needed qt half rides the second HWDGE engine
                # (ACT queue is empty before the first exp) so the two loads'
                # ~2us HBM completion latencies overlap
                qeng = nc.scalar if head == 0 else nc.sync
                qeng.dma_start(
                    out=qt_t[0:E, 0:LW],
                    in_=qt.ap()[head * E:(head + 1) * E, 0:LW])
                nc.gpsimd.memset(qt_t[E:128, 0:LW], 0.0)
                nc.sync.dma_start(
                    out=kt_t[0:E, 128:S],
                    in_=kt.ap()[head * E:(head + 1) * E, 128:S])
                nc.gpsimd.memset(kt_t[E:128, 128:S], 0.0)
                nc.sync.dma_start(
                    out=qt_t[0:E, LW:L],
                    in_=qt.ap()[head * E:(head + 1) * E, LW:L])
                nc.gpsimd.memset(qt_t[E:128, LW:L], 0.0)
                nc.sync.dma_start(
                    out=vt_t[:, :],
                    in_=vt.ap()[head * 128:(head + 1) * 128, :])
                o_t = o_pool.tile([64, L], f32, name="o_t", tag="o")
                heads[head] = (qt_t, kt_t, vt_t, o_t)

            def emit_mm1(head, lg, i):
                if lg == 0 and i == 0 and head not in heads:
                    load_head(head)
                if lg == 1 and i == 8 and head + 1 < HPC:
                    load_head(head + 1)
                if i == 0:
                    psos[(head, lg)] = ps_o_pool.tile(
                        [VW, LW], f32, name="ps_o", tag="ps_o")
                qt_t, kt_t, _, _ = heads[head]
                ps_s = ps_s_pool.tile([128, LW], f32, name="ps_s", tag="ps_s")
                for g in range(NG):
                    nc.tensor.matmul(
                        out=ps_s[:, g * 512:(g + 1) * 512],
                        lhsT=kt_t[:, i * 128:(i + 1) * 128],
                        rhs=qt_t[:, lg * LW + g * 512:lg * LW + (g + 1) * 512],
                        start=True, stop=True)
                return ps_s

            def emit_tail(head, lg, i, ps_s):
                qt_t, kt_t, vt_t, o_t = heads[head]
                ps_o = psos[(head, lg)]
                e_t = exp_pool.tile([128, LW], bf16, name="e_t", tag="e_t")
                if i in DVE_CHUNKS:
                    nc.vector.tensor_scalar(
                        out=e_t[:, :].bitcast(i16),
                        in0=ps_s[:, :],
                        scalar1=EXPA,
                        scalar2=EXPB,
                        op0=Mult,
                        op1=Add,
                    )
                else:
                    nc.scalar.activation(e_t[:, :], ps_s[:, :], Exp,
                                         scale=0.125)
                for g in range(NG):
                    nc.tensor.matmul(
                        out=ps_o[:, g * 512:(g + 1) * 512],
                        lhsT=vt_t[:, i * VW:(i + 1) * VW],
                        rhs=e_t[:, g * 512:(g + 1) * 512],
                        start=(i == 0), stop=(i == NCH - 1))
                if i == NCH - 1:
                    final = (head == HPC - 1 and lg == LG - 1)
                    if not final:
                        # Copy U^T and Z out of PSUM first: ps_o is
                        # single-buffered, so the next lg's first MM2
                        # (start=True overwrite) waits only on these two
                        # copies (~2.4us) instead of the whole norm chain.
                        # Z lands in a partition-0 tile: the custom-DVE
                        # fast reciprocal misreads offset-partition APs.
                        u_t = nrm_pool.tile([64, LW], f32, name="u",
                                            tag="u")
                        nc.vector.tensor_copy(u_t[:, :], ps_o[0:64, :])
                        zc_t = nrm_pool.tile([1, LW], f32, name="zc",
                                             tag="zc")
                        nc.vector.tensor_copy(zc_t[:, :], ps_o[64:65, :])
                        recip_t = nrm_pool.tile([1, LW], f32, name="re",
                                                tag="recip")
                        nc.vector.reciprocal_approx_fast(recip_t[:, :],
                                                         zc_t[:, :])
                        bcast_t = nrm_pool.tile([64, LW], f32, name="bc",
                                                tag="bcast")
                        nc.gpsimd.partition_broadcast(bcast_t[:, :],
                                                      recip_t[:, :],
                                                      channels=64)
                        nc.vector.tensor_mul(o_t[:, lg * LW:(lg + 1) * LW],
                                             u_t[:, :], bcast_t[:, :])
                        nc.gpsimd.dma_start(
                            out=out.ap()[head * 64:(head + 1) * 64,
                                         lg * LW:(lg + 1) * LW],
                            in_=o_t[:, lg * LW:(lg + 1) * LW])
                        return
                    # final tile: split into 512-col half-chains, emission
                    # interleaved so the DVE FIFO never blocks on gpsimd
                    halves = ((0, 512), (512, LW))
                    rts = []
                    for p, (c0, c1) in enumerate(halves):
                        w = c1 - c0
                        zc_t = nrm_pool.tile([1, w], f32, name=f"fzc{p}",
                                             tag=f"fzc{p}")
                        nc.scalar.copy(zc_t[:, :], ps_o[64:65, c0:c1])
                        recip_t = nrm_pool.tile([1, w], f32, name=f"fre{p}",
                                                tag=f"fre{p}")
                        nc.vector.reciprocal_approx_fast(recip_t[:, :],
                                                         zc_t[:, :])
                        rts.append(recip_t)
                    for p, (c0, c1) in enumerate(halves):
                        w = c1 - c0
                        bcast_t = nrm_pool.tile([64, w], f32, name=f"fbc{p}",
                                                tag=f"fbc{p}")
                        nc.gpsimd.partition_broadcast(bcast_t[:, :],
                                                      rts[p][:, :],
                                                      channels=64)
                        nc.vector.tensor_mul(
                            o_t[:, lg * LW + c0:lg * LW + c1],
                            ps_o[0:64, c0:c1], bcast_t[:, :])
                        nc.sync.dma_start(
                            out=out.ap()[head * 64:(head + 1) * 64,
                                         lg * LW + c0:lg * LW + c1],
                            in_=o_t[:, lg * LW + c0:lg * LW + c1])

            slots = [(head, lg, i)
                     for head in range(HPC)
                     for lg in range(LG)
                     for i in range(NCH)]
            # head 0's loads FIRST: the scalar-engine qt DMA must issue
            # before the warm exp's ACT table load occupies that queue
            load_head(0)
            # warm the ACT exp table set during the load ramp
            warm_t = nrm_pool.tile([1, 8], f32, tag="warm")
            nc.vector.memset(warm_t[:, :], 0.0)
            nc.scalar.activation(warm_t[:, :], warm_t[:, :], Exp, scale=1.0)

            pend = []
            for head, lg, i in slots:
                ps_s = emit_mm1(head, lg, i)
                pend.append((head, lg, i, ps_s))
                if len(pend) > 2:
                    emit_tail(*pend.pop(0))
            while pend:
                emit_tail(*pend.pop(0))
    nc.compile()
    return nc


def _prep_inputs(queries, keys, values):
    import ml_dtypes

    bf = ml_dtypes.bfloat16
    q = np.asarray(queries, dtype=np.float32)
    k = np.asarray(keys, dtype=np.float32)
    v = np.asarray(values, dtype=np.float32)
    BH = B * H
    qt = np.ascontiguousarray(q.transpose(0, 2, 3, 1)).astype(bf).reshape(
        BH, E, L)
    kt = np.ascontiguousarray(k.transpose(0, 2, 3, 1)).astype(bf).reshape(
        BH, E, S)
    vp = np.concatenate([v, np.ones((B, S, H, 1), np.float32)], axis=3)
    vt = (np.ascontiguousarray(
            vp.transpose(0, 2, 1, 3)
              .reshape(BH, NCH, 128, VW)
              .transpose(0, 2, 1, 3))
          .astype(bf)
          .reshape(BH, 128, NCH * VW))
    in_maps = []
    for c in range(N_CORES):
        sl = slice(c * HPC, (c + 1) * HPC)
        in_maps.append({
            "qt": np.ascontiguousarray(qt[sl]).reshape(HPC * E, L),
            "kt": np.ascontiguousarray(kt[sl]).reshape(HPC * E, S),
            "vt": np.ascontiguousarray(vt[sl]).reshape(HPC * 128, NCH * VW),
        })
    return in_maps


def _run(queries, keys, values, trace=False):
    global _compiled
    from concourse.bass_utils import run_bass_kernel_spmd

    if _compiled is None:
        _compiled = _build()
    in_maps = _prep_inputs(queries, keys, values)
    res = run_bass_kernel_spmd(_compiled, in_maps,
                               core_ids=list(range(N_CORES)), trace=trace)
    outs = np.stack([res.results[c]["out"] for c in range(N_CORES)])
    full = (outs.reshape(B * H, D, L)
                .reshape(B, H, D, L)
                .transpose(0, 3, 1, 2))
    return np.ascontiguousarray(full), res.exec_time_ns


def kernel(queries, keys, values):
    out, _ = _run(queries, keys, values, trace=False)
    return out

